# revision 13
# baseline (speedup 1.0000x reference)
"""Trainium2 Bass kernel for nn_Branch_Cell (branched LSTM-style cell).

Sharding: expert parallelism over the naxis dimension (naxis == 8 == n_cores).
Core `a` owns axis `a`: it streams that axis's ~189 MB of weights from HBM
(the memory roofline), computes h_m[a], c_m[a] and the partial sums
t1[a], t2[a]; the host sums t1/t2 over axes and applies the final
sigmoid*tanh (8 KB of work, avoids the collective latency floor).

Precision modes:
  'hybrid' (default): weights stream as host-decomposed bf16 hi + bf16 lo
     pairs (same total bytes as fp32; PE runs both halves at full bf16 rate
     with the vector's (hi, lo) pair as a [K, 2] stationary operand) --
     EXCEPT gate 1, whose output feeds a division (1 - |(xg1-hg1)/xg1|)
     that amplifies error near xg1 ~ 0; that gate streams exact fp32.
  'bf16x2': everything bf16 hi/lo.
  'f32': everything fp32 (PE at 4 cyc/row, slightly under HBM rate).

All matvecs keep the vector as the PE stationary operand (weights are the
moving operand), so kernel time is bounded by HBM->SBUF DMA. Weight blocks
are [512 rows x colspan cols] laid out [128p, 4kc, colspan] so every DMA
descriptor moves a 2 KB contiguous run (~95% of DMA line rate).
Elementwise gate/LN math runs on single-partition rows (ACT + DVE);
row->partition-chunk conversion for downstream lhsT operands uses tiny
K=1 transpose matmuls on the PE (no DMA scatters).
"""

import numpy as np
import ml_dtypes
from contextlib import ExitStack

import concourse.bass as bass
import concourse.tile as tile
from concourse import mybir
from concourse.bass_utils import run_bass_kernel_spmd

F32 = mybir.dt.float32
F32R = mybir.dt.float32r
BF16 = mybir.dt.bfloat16
AF = mybir.ActivationFunctionType
ALU = mybir.AluOpType

P = 128
INP = 2048
HID = 2048
AHID = 1024
NAXIS = 8
NT = 512                 # matmul moving free dim
KC_H = HID // P          # 16
KC_A = AHID // P         # 8

MODE = "hybrid"          # "hybrid" | "hybrid_r" | "f32r" | "bf16x2" | "f32"

# Weight dtypes per precision name
_WDT = None  # set below after mybir import resolution

import os as _os
if _os.environ.get("LDWOPT", "0") == "1":
    # walrus's own default is --enable-ldw-opt=true; bass pins it false.
    # Allow re-enabling for A/B (validated against the f64 oracle).
    import concourse.bass_utils as _bu
    _orig_run_command = _bu.run_command
    def _run_command_ldwopt(argv, **kw):
        argv = ["--enable-ldw-opt=true" if a == "--enable-ldw-opt=false" else a
                for a in argv]
        return _orig_run_command(argv, **kw)
    _bu.run_command = _run_command_ldwopt

EPS = 1e-5


# ---------------------------------------------------------------------------
# Workaround: the nix walrus in this container allows only ONE sync wait per
# non-EventSemaphore instruction ("Too many sync wait commands" in
# setupSyncWait). Tile's kernel-tail drain (and occasionally other insts)
# carries several. Split the extras onto single-wait NoOps placed just before
# the instruction on the same engine (per-engine program order preserved).
# ---------------------------------------------------------------------------
def _split_multi_waits(nc):
    n_new = 0
    for f in nc.m.functions:
        for blk in f.blocks:
            out = []
            for inst in blk.instructions:
                si = inst.sync_info
                waits = list(si.on_wait) if si is not None else []
                if len(waits) > 1 and inst.opcode != "EventSemaphore":
                    for w in waits[:-1]:
                        n_new += 1
                        out.append(
                            mybir.InstNoOp(
                                name=f"{inst.name}-wsplit{n_new}",
                                engine=inst.engine,
                                debug=inst.debug,
                                ins=[],
                                outs=[],
                                sync_info=mybir.SyncInfo(on_update=[], on_wait=[w]),
                            )
                        )
                    si.on_wait = [waits[-1]]
                    inst.sync_info = si
                out.append(inst)
            blk.instructions = out
    return n_new


def _gate_prec(mode, g):
    if mode == "f32":
        return "f32"
    if mode == "f32r":
        return "f32r"
    if mode in ("hybrid", "hybrid_r") and g == 1:
        return "f32"
    return "f32r" if mode == "hybrid_r" else "bf16"


def _prec_b(mode):
    if mode == "f32":
        return "f32"
    if mode in ("f32r", "hybrid_r"):
        return "f32r"
    return "bf16"


# ---------------------------------------------------------------------------
# Kernel builder
# ---------------------------------------------------------------------------
def _build(mode):
    nc = bass.Bass()
    prec_b = _prec_b(mode)

    def din(name, shape, dt=F32):
        return nc.dram_tensor(name, shape, dt, kind="ExternalInput")

    def dout(name, shape, dt=F32):
        return nc.dram_tensor(name, shape, dt, kind="ExternalOutput")

    def w_pair(name, shape, prec):
        """Declare weight tensor(s): (hi_handle, lo_handle|None)."""
        if prec == "bf16":
            return din(name + "_hi", shape, BF16), din(name + "_lo", shape, BF16)
        return din(name, shape, F32 if prec == "f32" else F32R), None

    gate_prec = {g: _gate_prec(mode, g) for g in range(4)}
    Wx_t = {g: w_pair(f"Wx{g}", [INP, HID], gate_prec[g]) for g in range(4)}
    Wh_t = {g: w_pair(f"Wh{g}", [HID, HID], gate_prec[g]) for g in range(4)}
    Wdbx_t = {j: w_pair(f"Wdbx{j}", [HID, AHID], prec_b) for j in range(3)}
    Wdbh_t = {j: w_pair(f"Wdbh{j}", [AHID, AHID], prec_b) for j in range(3)}
    Wict_t = w_pair("Wict", [HID, AHID], prec_b)
    Wilc_t = w_pair("Wilc", [AHID, AHID], prec_b)
    Wilh_t = w_pair("Wilh", [AHID, AHID], prec_b)

    need_bf_vec = any(p == "bf16" for p in gate_prec.values())
    need_f32_vec = any(p == "f32" for p in gate_prec.values())
    need_f32r_vec = any(p == "f32r" for p in gate_prec.values())

    xv_t = din("xvq", [P, KC_H, 2], BF16) if need_bf_vec else None
    hv_t = din("hvq", [P, KC_H, 2], BF16) if need_bf_vec else None
    xvf_t = din("xvf", [P, KC_H], F32) if need_f32_vec else None
    hvf_t = din("hvf", [P, KC_H], F32) if need_f32_vec else None
    xvr_t = din("xvr", [P, KC_H], F32R) if need_f32r_vec else None
    hvr_t = din("hvr", [P, KC_H], F32R) if need_f32r_vec else None
    if prec_b == "bf16":
        dv_t = din("dvq", [P, KC_A, 2], BF16)
    elif prec_b == "f32r":
        dv_t = din("dvr", [P, KC_A], F32R)
    else:
        dv_t = din("dvf", [P, KC_A], F32)

    c0_t = din("c0r", [1, HID])
    gb_t = din("gbc", [P, 4, KC_H])      # g0,b0,g1,b1 chunked [p, i, kc]
    bdb_t = din("bdbr", [1, 3, AHID])    # bdbx[:,a] + bdbh[:,a]
    bilc_t = din("bilcr", [1, AHID])
    bilh_t = din("bilhr", [1, AHID])

    hm_t = dout("hm", [1, HID])
    cm_t = dout("cm", [1, HID])
    t1_t = dout("t1", [1, AHID])
    t2_t = dout("t2", [1, AHID])

    with tile.TileContext(nc) as tc:
        with ExitStack() as ctx:
            wpool = ctx.enter_context(tc.tile_pool(name="wstream", bufs=6))
            rows = ctx.enter_context(tc.tile_pool(name="rows", bufs=1))
            stmps = ctx.enter_context(tc.tile_pool(name="stmps", bufs=3))
            kcp = ctx.enter_context(tc.tile_pool(name="kcp", bufs=1))
            sm = ctx.enter_context(tc.tile_pool(name="smalls", bufs=1))
            pmm = ctx.enter_context(tc.tile_pool(name="pmm", bufs=4, space="PSUM"))
            ptr = ctx.enter_context(tc.tile_pool(name="ptr", bufs=2, space="PSUM"))
            pcol = ctx.enter_context(tc.tile_pool(name="pcol", bufs=2, space="PSUM"))

            # ---- small input loads -----------------------------------
            xv = hv = xvf = hvf = xvr = hvr = None
            if need_bf_vec:
                xv = kcp.tile([P, KC_H, 2], BF16, tag="xv")
                hv = kcp.tile([P, KC_H, 2], BF16, tag="hv")
                nc.gpsimd.dma_start(xv[:], xv_t[:])
                nc.gpsimd.dma_start(hv[:], hv_t[:])
            if need_f32_vec:
                xvf = kcp.tile([P, KC_H], F32, tag="xvf")
                hvf = kcp.tile([P, KC_H], F32, tag="hvf")
                nc.gpsimd.dma_start(xvf[:], xvf_t[:])
                nc.gpsimd.dma_start(hvf[:], hvf_t[:])
            if need_f32r_vec:
                xvr = kcp.tile([P, KC_H], F32R, tag="xvr")
                hvr = kcp.tile([P, KC_H], F32R, tag="hvr")
                nc.gpsimd.dma_start(xvr[:], xvr_t[:])
                nc.gpsimd.dma_start(hvr[:], hvr_t[:])
            if prec_b == "bf16":
                dv = kcp.tile([P, KC_A, 2], BF16, tag="dv")
            elif prec_b == "f32r":
                dv = kcp.tile([P, KC_A], F32R, tag="dv")
            else:
                dv = kcp.tile([P, KC_A], F32, tag="dv")
            nc.gpsimd.dma_start(dv[:], dv_t[:])

            ones = sm.tile([1, 1], F32, tag="ones")
            nc.vector.memset(ones[:], 1.0)
            ones2 = sm.tile([2, 1], F32, tag="ones2")
            nc.vector.memset(ones2[:], 1.0)

            # ---- helpers ---------------------------------------------
            def collapse(ps, prec):
                """psum [M, 512] -> psum row [1, 512] AP (hi+lo summed).

                PSUM reads must start at partition 0 and a DVE op may read
                only one PSUM operand, so the two rows are combined on the
                PE: copy [2, NT] to SBUF, then ones2.T @ t sums the rows.
                """
                if prec != "bf16":
                    return ps[0:1, :]
                t = stmps.tile([2, NT], F32, tag="clp")
                nc.vector.tensor_copy(t[:], ps[:])
                pc = pcol.tile([1, NT], F32, tag="pcol")
                nc.tensor.matmul(pc[:], ones2[0:2, 0:1], t[:], start=True, stop=True)
                return pc

            # Consume-stage software pipeline: each column-block's psum
            # collapse/activation is deferred until the NEXT block's matmuls
            # have been emitted, so the in-order PE never stalls on the DVE
            # copy feeding the collapse matmul.
            deferred = []

            def flush_deferred():
                while deferred:
                    deferred.pop(0)()

            def stream_group(contribs, w_cols, prec, consume, scope):
                """Sum_i vec_i.T @ W_i over full K, streamed in row-contiguous
                blocks; calls consume(col_off, ps, prec) per 512-col output
                (deferred by one block).

                contribs: list of (vec_tile, (w_hi, w_lo), K).
                Weight blocks are [512, colspan] -> [128, 4, colspan] tiles
                (every DMA descriptor run = colspan * dtype = 2 KB).
                """
                colspan = 1024 if prec == "bf16" else 512
                n_ct = colspan // NT
                m_dim = 2 if prec == "bf16" else 1
                n_halves = 2 if prec == "bf16" else 1
                wdt = {"bf16": BF16, "f32": F32, "f32r": F32R}[prec]
                wtag = "wblk"
                wbufs = 8
                total_per_ct = sum((K // NT) * 4 * n_halves for _, _, K in contribs)

                for cb in range(w_cols // colspan):
                    with nc.named_scope(f"{scope}_cb{cb}"):
                        pss = [
                            pmm.tile([m_dim, NT], F32, tag="mmps", name="mmps")
                            for _ in range(n_ct)
                        ]
                        cnt = 0
                        n_blk = 0
                        for vec, (w_hi, w_lo), K in contribs:
                            for kb in range(K // NT):
                                for w_t in (w_hi, w_lo)[:n_halves]:
                                    blk = wpool.tile(
                                        [P, 4, colspan], wdt, tag=wtag, bufs=wbufs
                                    )
                                    nc.sync.dma_start(
                                        blk[:],
                                        w_t[
                                            kb * NT : (kb + 1) * NT,
                                            cb * colspan : (cb + 1) * colspan,
                                        ].rearrange("(kc p) n -> p kc n", p=P),
                                    )
                                    for kc in range(4):
                                        ki = kb * 4 + kc
                                        lhsT = (
                                            vec[:, ki, :]
                                            if prec == "bf16"
                                            else vec[:, ki : ki + 1]
                                        )
                                        for ct in range(n_ct):
                                            nc.tensor.matmul(
                                                pss[ct][:],
                                                lhsT,
                                                blk[:, kc, ct * NT : (ct + 1) * NT],
                                                start=(cnt // n_ct == 0),
                                                stop=(cnt // n_ct == total_per_ct - 1),
                                            )
                                            cnt += 1
                                    n_blk += 1
                                    if n_blk == 1:
                                        flush_deferred()

                        def _consume_cb(cb=cb, pss=pss):
                            for ct in range(n_ct):
                                consume(cb * colspan + ct * NT, pss[ct], prec)

                        deferred.append(_consume_cb)

            def pe_transpose(row_ap, cpt):
                """[1, cpt*128] f32 row -> psum tile [P, cpt] f32."""
                pst = ptr.tile([P, KC_H], F32, tag="ptr")
                for c in range(cpt):
                    nc.tensor.matmul(
                        pst[:, c : c + 1],
                        row_ap[0:1, c * P : (c + 1) * P],
                        ones[0:1, 0:1],
                        start=True,
                        stop=True,
                    )
                return pst

            def to_lhsT(src_ap, cpt, name):
                """[P, cpt] f32 (psum/sbuf) -> lhsT tile for stage-B matvecs."""
                if prec_b == "bf16":
                    tq = kcp.tile([P, cpt, 2], BF16, tag=name)
                    nc.vector.tensor_copy(tq[:, :, 0], src_ap)
                    hi32 = kcp.tile([P, cpt], F32, tag=name + "32")
                    nc.vector.tensor_copy(hi32[:], tq[:, :, 0])
                    nc.vector.tensor_sub(tq[:, :, 1], src_ap, hi32[:])
                    return tq
                wdt = F32R if prec_b == "f32r" else F32
                tq = kcp.tile([P, cpt], wdt, tag=name)
                nc.vector.tensor_copy(tq[:], src_ap)
                return tq

            def layernorm_lhsT(src_row, gb_idx, name):
                """LN over [1, HID] row; gamma/beta applied after the
                row->[P, KC_H] transpose. Returns stage-B lhsT tile."""
                stats = sm.tile([1, HID // 512, 6], F32, tag=name + "st")
                for c_ in range(HID // 512):
                    nc.vector.bn_stats(
                        stats[:, c_, :], src_row[:, c_ * 512 : (c_ + 1) * 512]
                    )
                mv = sm.tile([1, 2], F32, tag=name + "mv")
                nc.vector.bn_aggr(mv[:], stats[:])
                vs = sm.tile([1, 1], F32, tag=name + "vs")
                nc.vector.tensor_scalar_add(vs[:], mv[:, 1:2], EPS)
                sd = sm.tile([1, 1], F32, tag=name + "sd")
                nc.scalar.sqrt(sd[:], vs[:])
                inv = sm.tile([1, 1], F32, tag=name + "inv")
                nc.vector.reciprocal(inv[:], sd[:])
                nmu = sm.tile([1, 1], F32, tag=name + "nmu")
                nc.vector.scalar_tensor_tensor(
                    nmu[:], mv[:, 0:1], -1.0, inv[:], ALU.mult, ALU.mult
                )
                ln0 = rows.tile([1, HID], F32, tag="ln0")
                nc.scalar.activation(
                    ln0[:], src_row[:], AF.Identity, bias=nmu[:], scale=inv[:]
                )
                pst = pe_transpose(ln0, KC_H)
                lnc = kcp.tile([P, KC_H], F32, tag=name + "c")
                nc.vector.tensor_mul(lnc[:], pst[:, :KC_H], gbc[:, gb_idx, :])
                nc.vector.tensor_add(lnc[:], lnc[:], gbc[:, gb_idx + 1, :])
                return to_lhsT(lnc[:], KC_H, name + "kc")

            def gvec(prec):
                if prec == "bf16":
                    return (xv, hv)
                if prec == "f32r":
                    return (xvr, hvr)
                return (xvf, hvf)

            # ---- stage A: gates --------------------------------------
            ig_r = rows.tile([1, HID], F32, tag="ig")
            gg_r = rows.tile([1, HID], F32, tag="gg")
            og_r = rows.tile([1, HID], F32, tag="og")
            d_r = rows.tile([1, HID], F32, tag="dr")
            rc_r = rows.tile([1, HID], F32, tag="rc")

            gate_rows = {0: ig_r, 2: gg_r, 3: og_r}
            gate_fn = {0: AF.Sigmoid, 2: AF.Tanh, 3: AF.Sigmoid}

            def emit_gate(g):
                pg = gate_prec[g]
                xvec, hvec = gvec(pg)

                def mk_consume(gr=gate_rows[g], fn=gate_fn[g]):
                    def consume(col, ps, prec):
                        nc.scalar.activation(
                            gr[:, col : col + NT], collapse(ps, prec), fn
                        )
                    return consume

                stream_group(
                    [(xvec, Wx_t[g], INP), (hvec, Wh_t[g], HID)],
                    HID, pg, mk_consume(), f"g{g}",
                )

            def consume_x(col, ps, prec):
                pc = collapse(ps, prec)
                nc.vector.tensor_copy(d_r[:, col : col + NT], pc)
                nc.vector.reciprocal(rc_r[:, col : col + NT], pc)

            def consume_h(col, ps, prec):
                nc.vector.tensor_sub(
                    d_r[:, col : col + NT],
                    d_r[:, col : col + NT],
                    collapse(ps, prec),
                )

            pg1 = gate_prec[1]
            xv1, hv1 = gvec(pg1)
            # interleave the (PE-heavier) fp32 gate-1 passes between the
            # bf16 gates so the PE hump is absorbed by the DMA prefetch bufs
            emit_gate(0)
            # small loads not needed until much later: emitted here so their
            # DMA-queue slots come after the first weight blocks (faster start)
            c0r = rows.tile([1, HID], F32, tag="c0r")
            nc.gpsimd.dma_start(c0r[:], c0_t[:])
            gbc = kcp.tile([P, 4, KC_H], F32, tag="gbc")
            nc.gpsimd.dma_start(gbc[:], gb_t[:])
            bdbr = rows.tile([1, 3, AHID], F32, tag="bdbr")
            nc.gpsimd.dma_start(bdbr[:], bdb_t[:])
            bilcr = rows.tile([1, AHID], F32, tag="bilcr")
            nc.gpsimd.dma_start(bilcr[:], bilc_t[:])
            bilhr = rows.tile([1, AHID], F32, tag="bilhr")
            nc.gpsimd.dma_start(bilhr[:], bilh_t[:])
            stream_group([(xv1, Wx_t[1], INP)], HID, pg1, consume_x, "g1x")
            emit_gate(2)
            stream_group([(hv1, Wh_t[1], HID)], HID, pg1, consume_h, "g1h")
            emit_gate(3)

            # ---- stage A chain: c_m, h_m, layernorms ------------------
            flush_deferred()
            with nc.named_scope("gate_chain"):
                # q = d / xg1   (in place of d)
                nc.vector.tensor_mul(d_r[:], d_r[:], rc_r[:])
                # |q|  (in place of rc)
                nc.scalar.activation(rc_r[:], d_r[:], AF.Abs)
                # |q| * c0   (into d: q dead)
                nc.vector.tensor_mul(d_r[:], rc_r[:], c0r[:])
                # ig * gg  (in place of ig)
                nc.vector.tensor_mul(ig_r[:], ig_r[:], gg_r[:])
                # fg_p*c0 = c0 - |q|*c0   (into rc: |q| dead)
                nc.vector.tensor_sub(rc_r[:], c0r[:], d_r[:])
                cm_r = rows.tile([1, HID], F32, tag="cmr")
                nc.vector.tensor_add(cm_r[:], rc_r[:], ig_r[:])
                nc.gpsimd.dma_start(cm_t[:], cm_r[:])

            with nc.named_scope("ln_c"):
                lc_kc = layernorm_lhsT(cm_r, 2, "lnc")

            with nc.named_scope("h_m"):
                th_r = rows.tile([1, HID], F32, tag="gg")  # reuse gg slot
                nc.scalar.activation(th_r[:], cm_r[:], AF.Tanh)
                # h_m = og * tanh(c_m)  (in place of og)
                nc.vector.tensor_mul(og_r[:], og_r[:], th_r[:])
                nc.gpsimd.dma_start(hm_t[:], og_r[:])

            with nc.named_scope("ln_h"):
                ht_kc = layernorm_lhsT(og_r, 0, "lnh")

            # ---- stage B ---------------------------------------------
            ct_r = rows.tile([1, AHID], F32, tag="ctr")

            def consume_ct(col, ps, prec):
                nc.vector.tensor_copy(ct_r[:, col : col + NT], collapse(ps, prec))

            stream_group([(lc_kc, Wict_t, HID)], AHID, prec_b, consume_ct, "ict")

            indb_r = rows.tile([1, AHID], F32, tag="indb")
            fndb_r = rows.tile([1, AHID], F32, tag="fndb")
            cndb_r = rows.tile([1, AHID], F32, tag="cndb")
            pre_rows = {0: indb_r, 1: fndb_r, 2: cndb_r}
            pre_fn = {0: AF.Sigmoid, 1: AF.Sigmoid, 2: AF.Tanh}

            def emit_db(j):
                def mk_consume_pre(jj=j):
                    def consume(col, ps, prec):
                        srow = stmps.tile([1, NT], F32, tag="stmp2")
                        nc.vector.tensor_add(
                            srow[:], collapse(ps, prec),
                            bdbr[:, jj, col : col + NT],
                        )
                        nc.scalar.activation(
                            pre_rows[jj][:, col : col + NT], srow[:], pre_fn[jj]
                        )
                    return consume

                stream_group(
                    [(ht_kc, Wdbx_t[j], HID), (dv, Wdbh_t[j], AHID)],
                    AHID, prec_b, mk_consume_pre(), f"db{j}",
                )

            def mk_consume_il(bias_r, out_t):
                def consume(col, ps, prec):
                    srow = stmps.tile([1, NT], F32, tag="stmp3")
                    nc.vector.tensor_add(
                        srow[:], collapse(ps, prec), bias_r[:, col : col + NT]
                    )
                    nc.gpsimd.dma_start(out_t[:, col : col + NT], srow[:])
                return consume

            # ---- order: db0 -> i_cell/t1 overlap db1+db2 -> c_cell/t2 ----
            emit_db(0)
            flush_deferred()
            with nc.named_scope("cells1"):
                # i_cell = in_db * c_t (in place of indb)
                nc.vector.tensor_mul(indb_r[:], indb_r[:], ct_r[:])
                ic_kc = to_lhsT(pe_transpose(indb_r, KC_A)[:, :KC_A], KC_A, "ickc")
            stream_group(
                [(ic_kc, Wilc_t, AHID)], AHID, prec_b,
                mk_consume_il(bilcr, t1_t), "t1",
            )
            emit_db(1)
            emit_db(2)
            flush_deferred()
            with nc.named_scope("cells2"):
                # fn_db * cn_db (in place of fndb)
                nc.vector.tensor_mul(fndb_r[:], fndb_r[:], cndb_r[:])
                # c_cell (in place of cndb)
                nc.vector.tensor_add(cndb_r[:], fndb_r[:], indb_r[:])
                cc_kc = to_lhsT(pe_transpose(cndb_r, KC_A)[:, :KC_A], KC_A, "cckc")
            stream_group(
                [(cc_kc, Wilh_t, AHID)], AHID, prec_b,
                mk_consume_il(bilhr, t2_t), "t2",
            )
            flush_deferred()

    _split_multi_waits(nc)
    return nc


_NC_CACHE = {}


def _get_nc(mode):
    if mode not in _NC_CACHE:
        _NC_CACHE[mode] = _build(mode)
    return _NC_CACHE[mode]


# ---------------------------------------------------------------------------
# Host side: shard, run, gather
# ---------------------------------------------------------------------------
def _bf16_split(w):
    """fp32 array -> (hi, lo) bf16 arrays with hi + lo ~= w (round-to-nearest).

    Bit-twiddled for speed: hi = RNE-round to bf16; lo = RNE(w - hi).
    """
    w = np.ascontiguousarray(w, dtype=np.float32)
    u = w.view(np.uint32)
    rhi = (u + 0x7FFF + ((u >> 16) & 1)) & 0xFFFF0000
    hi32 = rhi.view(np.float32)
    hi = (rhi >> 16).astype(np.uint16).view(ml_dtypes.bfloat16)
    lo = (w - hi32).astype(ml_dtypes.bfloat16)
    return np.ascontiguousarray(hi), np.ascontiguousarray(lo)


def _chunk_vec(v, kc):
    """[kc*128] -> [128, kc] with elem [p, c] = v[c*128 + p]."""
    return np.ascontiguousarray(np.asarray(v, np.float32).reshape(kc, P).T)


def _vec_pair(v, kc):
    hi, lo = _bf16_split(np.asarray(v, np.float32).reshape(-1))
    q = np.empty((P, kc, 2), dtype=ml_dtypes.bfloat16)
    q[:, :, 0] = np.asarray(hi).reshape(kc, P).T
    q[:, :, 1] = np.asarray(lo).reshape(kc, P).T
    return q


def _prep_core_inputs(a, mode, ins, bdb):
    f32 = np.float32
    m = {}
    gate_prec = {g: _gate_prec(mode, g) for g in range(4)}
    prec_b = _prec_b(mode)

    def put_w(name, w, prec):
        w = np.ascontiguousarray(w, dtype=f32)
        if prec == "bf16":
            hi, lo = _bf16_split(w)
            m[name + "_hi"] = hi
            m[name + "_lo"] = lo
        else:
            m[name] = w

    for g in range(4):
        put_w(f"Wx{g}", ins["Wx"][g, a], gate_prec[g])
        put_w(f"Wh{g}", ins["Wh"][g, a], gate_prec[g])
    for j in range(3):
        put_w(f"Wdbx{j}", ins["Wdbx"][j, a], prec_b)
        put_w(f"Wdbh{j}", ins["Wdbh"][j, a], prec_b)
    put_w("Wict", ins["W_ict"][a], prec_b)
    put_w("Wilc", ins["W_ilc"][a], prec_b)
    put_w("Wilh", ins["W_ilh"][a], prec_b)

    need_bf_vec = any(p == "bf16" for p in gate_prec.values())
    need_f32_vec = any(p == "f32" for p in gate_prec.values())
    need_f32r_vec = any(p == "f32r" for p in gate_prec.values())
    if need_bf_vec:
        m["xvq"] = _vec_pair(ins["in_"], KC_H)
        m["hvq"] = _vec_pair(ins["h0"][a], KC_H)
    if need_f32_vec:
        m["xvf"] = _chunk_vec(ins["in_"], KC_H)
        m["hvf"] = _chunk_vec(ins["h0"][a], KC_H)
    if need_f32r_vec:
        m["xvr"] = _chunk_vec(ins["in_"], KC_H)
        m["hvr"] = _chunk_vec(ins["h0"][a], KC_H)
    if prec_b == "bf16":
        m["dvq"] = _vec_pair(ins["hdb0"], KC_A)
    elif prec_b == "f32r":
        m["dvr"] = _chunk_vec(ins["hdb0"], KC_A)
    else:
        m["dvf"] = _chunk_vec(ins["hdb0"], KC_A)

    m["c0r"] = np.ascontiguousarray(ins["c0"][a], dtype=f32).reshape(1, HID)
    gb = np.empty((P, 4, KC_H), dtype=f32)
    gb[:, 0] = _chunk_vec(ins["ln_gamma"][0, a], KC_H)
    gb[:, 1] = _chunk_vec(ins["ln_beta"][0, a], KC_H)
    gb[:, 2] = _chunk_vec(ins["ln_gamma"][1, a], KC_H)
    gb[:, 3] = _chunk_vec(ins["ln_beta"][1, a], KC_H)
    m["gbc"] = gb
    m["bdbr"] = np.ascontiguousarray(bdb[:, a], dtype=f32).reshape(1, 3, AHID)
    m["bilcr"] = np.ascontiguousarray(ins["b_ilc"][a], dtype=f32).reshape(1, AHID)
    m["bilhr"] = np.ascontiguousarray(ins["b_ilh"][a], dtype=f32).reshape(1, AHID)
    return m


def _run(inputs, mode=None, **run_kwargs):
    mode = mode or MODE
    f32 = np.float32
    ins = {k: np.asarray(v) for k, v in inputs.items()}
    bdb = ins["bdbx"].astype(f32) + ins["bdbh"].astype(f32)

    in_maps = [_prep_core_inputs(a, mode, ins, bdb) for a in range(NAXIS)]

    nc = _get_nc(mode)
    res = run_bass_kernel_spmd(nc, in_maps, core_ids=list(range(NAXIS)), **run_kwargs)

    h_m = np.stack([res.results[a]["hm"].reshape(HID) for a in range(NAXIS)])
    c_m = np.stack([res.results[a]["cm"].reshape(HID) for a in range(NAXIS)])
    t1 = np.stack([res.results[a]["t1"].reshape(AHID) for a in range(NAXIS)])
    t2 = np.stack([res.results[a]["t2"].reshape(AHID) for a in range(NAXIS)])

    su_c = t1.astype(np.float64).sum(0)
    su_h = t2.astype(np.float64).sum(0)
    h_db = (1.0 / (1.0 + np.exp(-su_h))) * np.tanh(su_c)

    return (h_m.astype(f32), c_m.astype(f32), h_db.astype(f32)), res


def kernel(**inputs):
    out, _ = _run(inputs)
    return out


# revision 14
# speedup vs baseline: 1.0864x; 1.0864x over previous
"""Trainium2 Bass kernel for nn_Branch_Cell (branched LSTM-style cell).

Sharding: expert parallelism over the naxis dimension (naxis == 8 == n_cores).
Core `a` owns axis `a`: it streams that axis's ~189 MB of weights from HBM
(the memory roofline), computes h_m[a], c_m[a] and the partial sums
t1[a], t2[a]; the host sums t1/t2 over axes and applies the final
sigmoid*tanh (8 KB of work, avoids the collective latency floor).

Precision modes:
  'hybrid' (default): weights stream as host-decomposed bf16 hi + bf16 lo
     pairs (same total bytes as fp32; PE runs both halves at full bf16 rate
     with the vector's (hi, lo) pair as a [K, 2] stationary operand) --
     EXCEPT gate 1, whose output feeds a division (1 - |(xg1-hg1)/xg1|)
     that amplifies error near xg1 ~ 0; that gate streams exact fp32.
  'bf16x2': everything bf16 hi/lo.
  'f32': everything fp32 (PE at 4 cyc/row, slightly under HBM rate).

All matvecs keep the vector as the PE stationary operand (weights are the
moving operand), so kernel time is bounded by HBM->SBUF DMA. Weight blocks
are [512 rows x colspan cols] laid out [128p, 4kc, colspan] so every DMA
descriptor moves a 2 KB contiguous run (~95% of DMA line rate).
Elementwise gate/LN math runs on single-partition rows (ACT + DVE);
row->partition-chunk conversion for downstream lhsT operands uses tiny
K=1 transpose matmuls on the PE (no DMA scatters).
"""

import numpy as np
import ml_dtypes
from contextlib import ExitStack

import concourse.bass as bass
import concourse.tile as tile
from concourse import mybir
from concourse.bass_utils import run_bass_kernel_spmd

F32 = mybir.dt.float32
F32R = mybir.dt.float32r
BF16 = mybir.dt.bfloat16
AF = mybir.ActivationFunctionType
ALU = mybir.AluOpType

P = 128
INP = 2048
HID = 2048
AHID = 1024
NAXIS = 8
NT = 512                 # matmul moving free dim
KC_H = HID // P          # 16
KC_A = AHID // P         # 8

MODE = "hybrid"          # "hybrid" | "hybrid_r" | "f32r" | "bf16x2" | "f32"

# Weight dtypes per precision name
_WDT = None  # set below after mybir import resolution

import os as _os
if _os.environ.get("LDWOPT", "0") == "1":
    # walrus's own default is --enable-ldw-opt=true; bass pins it false.
    # Allow re-enabling for A/B (validated against the f64 oracle).
    import concourse.bass_utils as _bu
    _orig_run_command = _bu.run_command
    def _run_command_ldwopt(argv, **kw):
        argv = ["--enable-ldw-opt=true" if a == "--enable-ldw-opt=false" else a
                for a in argv]
        return _orig_run_command(argv, **kw)
    _bu.run_command = _run_command_ldwopt

EPS = 1e-5


# ---------------------------------------------------------------------------
# Workaround: the nix walrus in this container allows only ONE sync wait per
# non-EventSemaphore instruction ("Too many sync wait commands" in
# setupSyncWait). Tile's kernel-tail drain (and occasionally other insts)
# carries several. Split the extras onto single-wait NoOps placed just before
# the instruction on the same engine (per-engine program order preserved).
# ---------------------------------------------------------------------------
def _split_multi_waits(nc):
    n_new = 0
    for f in nc.m.functions:
        for blk in f.blocks:
            out = []
            for inst in blk.instructions:
                si = inst.sync_info
                waits = list(si.on_wait) if si is not None else []
                if len(waits) > 1 and inst.opcode != "EventSemaphore":
                    for w in waits[:-1]:
                        n_new += 1
                        out.append(
                            mybir.InstNoOp(
                                name=f"{inst.name}-wsplit{n_new}",
                                engine=inst.engine,
                                debug=inst.debug,
                                ins=[],
                                outs=[],
                                sync_info=mybir.SyncInfo(on_update=[], on_wait=[w]),
                            )
                        )
                    si.on_wait = [waits[-1]]
                    inst.sync_info = si
                out.append(inst)
            blk.instructions = out
    return n_new


def _gate_prec(mode, g):
    if mode == "f32":
        return "f32"
    if mode == "f32r":
        return "f32r"
    if mode in ("hybrid", "hybrid_r") and g == 1:
        return "f32"
    return "f32r" if mode == "hybrid_r" else "bf16"


def _prec_b(mode):
    if mode == "f32":
        return "f32"
    if mode in ("f32r", "hybrid_r"):
        return "f32r"
    return "bf16"


# ---------------------------------------------------------------------------
# Kernel builder
# ---------------------------------------------------------------------------
def _build(mode):
    nc = bass.Bass()
    prec_b = _prec_b(mode)

    def din(name, shape, dt=F32):
        return nc.dram_tensor(name, shape, dt, kind="ExternalInput")

    def dout(name, shape, dt=F32):
        return nc.dram_tensor(name, shape, dt, kind="ExternalOutput")

    def w_pair(name, shape, prec):
        """Declare weight tensor(s): (hi_handle, lo_handle|None)."""
        if prec == "bf16":
            return din(name + "_hi", shape, BF16), din(name + "_lo", shape, BF16)
        return din(name, shape, F32 if prec == "f32" else F32R), None

    gate_prec = {g: _gate_prec(mode, g) for g in range(4)}
    Wx_t = {g: w_pair(f"Wx{g}", [INP, HID], gate_prec[g]) for g in range(4)}
    Wh_t = {g: w_pair(f"Wh{g}", [HID, HID], gate_prec[g]) for g in range(4)}
    Wdbx_t = {j: w_pair(f"Wdbx{j}", [HID, AHID], prec_b) for j in range(3)}
    Wdbh_t = {j: w_pair(f"Wdbh{j}", [AHID, AHID], prec_b) for j in range(3)}
    Wict_t = w_pair("Wict", [HID, AHID], prec_b)
    Wilc_t = w_pair("Wilc", [AHID, AHID], prec_b)
    Wilh_t = w_pair("Wilh", [AHID, AHID], prec_b)

    need_bf_vec = any(p == "bf16" for p in gate_prec.values())
    need_f32_vec = any(p == "f32" for p in gate_prec.values())
    need_f32r_vec = any(p == "f32r" for p in gate_prec.values())

    xv_t = din("xvq", [P, KC_H, 2], BF16) if need_bf_vec else None
    hv_t = din("hvq", [P, KC_H, 2], BF16) if need_bf_vec else None
    xvf_t = din("xvf", [P, KC_H], F32) if need_f32_vec else None
    hvf_t = din("hvf", [P, KC_H], F32) if need_f32_vec else None
    xvr_t = din("xvr", [P, KC_H], F32R) if need_f32r_vec else None
    hvr_t = din("hvr", [P, KC_H], F32R) if need_f32r_vec else None
    if prec_b == "bf16":
        dv_t = din("dvq", [P, KC_A, 2], BF16)
    elif prec_b == "f32r":
        dv_t = din("dvr", [P, KC_A], F32R)
    else:
        dv_t = din("dvf", [P, KC_A], F32)

    c0_t = din("c0r", [1, HID])
    gb_t = din("gbc", [P, 4, KC_H])      # g0,b0,g1,b1 chunked [p, i, kc]
    bdb_t = din("bdbr", [1, 3, AHID])    # bdbx[:,a] + bdbh[:,a]
    bilc_t = din("bilcr", [1, AHID])
    bilh_t = din("bilhr", [1, AHID])

    hm_t = dout("hm", [1, HID])
    cm_t = dout("cm", [1, HID])
    t1_t = dout("t1", [1, AHID])
    t2_t = dout("t2", [1, AHID])

    with tile.TileContext(nc) as tc:
        with ExitStack() as ctx:
            wpool = ctx.enter_context(tc.tile_pool(name="wstream", bufs=6))
            rows = ctx.enter_context(tc.tile_pool(name="rows", bufs=1))
            stmps = ctx.enter_context(tc.tile_pool(name="stmps", bufs=3))
            kcp = ctx.enter_context(tc.tile_pool(name="kcp", bufs=1))
            sm = ctx.enter_context(tc.tile_pool(name="smalls", bufs=1))
            pmm = ctx.enter_context(tc.tile_pool(name="pmm", bufs=4, space="PSUM"))
            ptr = ctx.enter_context(tc.tile_pool(name="ptr", bufs=2, space="PSUM"))
            pcol = ctx.enter_context(tc.tile_pool(name="pcol", bufs=2, space="PSUM"))

            # ---- small input loads -----------------------------------
            xv = hv = xvf = hvf = xvr = hvr = None
            if need_bf_vec:
                xv = kcp.tile([P, KC_H, 2], BF16, tag="xv")
                hv = kcp.tile([P, KC_H, 2], BF16, tag="hv")
                nc.scalar.dma_start(xv[:], xv_t[:])
                nc.scalar.dma_start(hv[:], hv_t[:])
            if need_f32_vec:
                xvf = kcp.tile([P, KC_H], F32, tag="xvf")
                hvf = kcp.tile([P, KC_H], F32, tag="hvf")
                nc.scalar.dma_start(xvf[:], xvf_t[:])
                nc.scalar.dma_start(hvf[:], hvf_t[:])
            if need_f32r_vec:
                xvr = kcp.tile([P, KC_H], F32R, tag="xvr")
                hvr = kcp.tile([P, KC_H], F32R, tag="hvr")
                nc.scalar.dma_start(xvr[:], xvr_t[:])
                nc.scalar.dma_start(hvr[:], hvr_t[:])
            if prec_b == "bf16":
                dv = kcp.tile([P, KC_A, 2], BF16, tag="dv")
            elif prec_b == "f32r":
                dv = kcp.tile([P, KC_A], F32R, tag="dv")
            else:
                dv = kcp.tile([P, KC_A], F32, tag="dv")
            nc.scalar.dma_start(dv[:], dv_t[:])

            ones = sm.tile([1, 1], F32, tag="ones")
            nc.vector.memset(ones[:], 1.0)
            ones2 = sm.tile([2, 1], F32, tag="ones2")
            nc.vector.memset(ones2[:], 1.0)

            # ---- helpers ---------------------------------------------
            def collapse(ps, prec):
                """psum [M, 512] -> psum row [1, 512] AP (hi+lo summed).

                PSUM reads must start at partition 0 and a DVE op may read
                only one PSUM operand, so the two rows are combined on the
                PE: copy [2, NT] to SBUF, then ones2.T @ t sums the rows.
                """
                if prec != "bf16":
                    return ps[0:1, :]
                t = stmps.tile([2, NT], F32, tag="clp")
                nc.vector.tensor_copy(t[:], ps[:])
                pc = pcol.tile([1, NT], F32, tag="pcol")
                nc.tensor.matmul(pc[:], ones2[0:2, 0:1], t[:], start=True, stop=True)
                return pc

            # Consume-stage software pipeline: each column-block's psum
            # collapse/activation is deferred until the NEXT block's matmuls
            # have been emitted, so the in-order PE never stalls on the DVE
            # copy feeding the collapse matmul.
            deferred = []

            def flush_deferred():
                while deferred:
                    deferred.pop(0)()

            def stream_group(contribs, w_cols, prec, consume, scope):
                """Sum_i vec_i.T @ W_i over full K, streamed in row-contiguous
                blocks; calls consume(col_off, ps, prec) per 512-col output
                (deferred by one block).

                contribs: list of (vec_tile, (w_hi, w_lo), K).
                Weight blocks are [512, colspan] -> [128, 4, colspan] tiles
                (every DMA descriptor run = colspan * dtype = 2 KB).
                """
                colspan = 1024 if prec == "bf16" else 512
                n_ct = colspan // NT
                m_dim = 2 if prec == "bf16" else 1
                n_halves = 2 if prec == "bf16" else 1
                wdt = {"bf16": BF16, "f32": F32, "f32r": F32R}[prec]
                wtag = "wblk"
                wbufs = 8
                total_per_ct = sum((K // NT) * 4 * n_halves for _, _, K in contribs)

                for cb in range(w_cols // colspan):
                    with nc.named_scope(f"{scope}_cb{cb}"):
                        pss = [
                            pmm.tile([m_dim, NT], F32, tag="mmps", name="mmps")
                            for _ in range(n_ct)
                        ]
                        cnt = 0
                        n_blk = 0
                        for vec, (w_hi, w_lo), K in contribs:
                            for kb in range(K // NT):
                                for w_t in (w_hi, w_lo)[:n_halves]:
                                    blk = wpool.tile(
                                        [P, 4, colspan], wdt, tag=wtag, bufs=wbufs
                                    )
                                    nc.sync.dma_start(
                                        blk[:],
                                        w_t[
                                            kb * NT : (kb + 1) * NT,
                                            cb * colspan : (cb + 1) * colspan,
                                        ].rearrange("(kc p) n -> p kc n", p=P),
                                    )
                                    for kc in range(4):
                                        ki = kb * 4 + kc
                                        lhsT = (
                                            vec[:, ki, :]
                                            if prec == "bf16"
                                            else vec[:, ki : ki + 1]
                                        )
                                        for ct in range(n_ct):
                                            nc.tensor.matmul(
                                                pss[ct][:],
                                                lhsT,
                                                blk[:, kc, ct * NT : (ct + 1) * NT],
                                                start=(cnt // n_ct == 0),
                                                stop=(cnt // n_ct == total_per_ct - 1),
                                            )
                                            cnt += 1
                                    n_blk += 1
                                    if n_blk == 1:
                                        flush_deferred()

                        def _consume_cb(cb=cb, pss=pss):
                            for ct in range(n_ct):
                                consume(cb * colspan + ct * NT, pss[ct], prec)

                        deferred.append(_consume_cb)

            def pe_transpose(row_ap, cpt):
                """[1, cpt*128] f32 row -> psum tile [P, cpt] f32."""
                pst = ptr.tile([P, KC_H], F32, tag="ptr")
                for c in range(cpt):
                    nc.tensor.matmul(
                        pst[:, c : c + 1],
                        row_ap[0:1, c * P : (c + 1) * P],
                        ones[0:1, 0:1],
                        start=True,
                        stop=True,
                    )
                return pst

            def to_lhsT(src_ap, cpt, name):
                """[P, cpt] f32 (psum/sbuf) -> lhsT tile for stage-B matvecs."""
                if prec_b == "bf16":
                    tq = kcp.tile([P, cpt, 2], BF16, tag=name)
                    nc.vector.tensor_copy(tq[:, :, 0], src_ap)
                    hi32 = kcp.tile([P, cpt], F32, tag=name + "32")
                    nc.vector.tensor_copy(hi32[:], tq[:, :, 0])
                    nc.vector.tensor_sub(tq[:, :, 1], src_ap, hi32[:])
                    return tq
                wdt = F32R if prec_b == "f32r" else F32
                tq = kcp.tile([P, cpt], wdt, tag=name)
                nc.vector.tensor_copy(tq[:], src_ap)
                return tq

            def layernorm_lhsT(src_row, gb_idx, name):
                """LN over [1, HID] row; gamma/beta applied after the
                row->[P, KC_H] transpose. Returns stage-B lhsT tile."""
                stats = sm.tile([1, HID // 512, 6], F32, tag=name + "st")
                for c_ in range(HID // 512):
                    nc.vector.bn_stats(
                        stats[:, c_, :], src_row[:, c_ * 512 : (c_ + 1) * 512]
                    )
                mv = sm.tile([1, 2], F32, tag=name + "mv")
                nc.vector.bn_aggr(mv[:], stats[:])
                vs = sm.tile([1, 1], F32, tag=name + "vs")
                nc.vector.tensor_scalar_add(vs[:], mv[:, 1:2], EPS)
                sd = sm.tile([1, 1], F32, tag=name + "sd")
                nc.scalar.sqrt(sd[:], vs[:])
                inv = sm.tile([1, 1], F32, tag=name + "inv")
                nc.vector.reciprocal(inv[:], sd[:])
                nmu = sm.tile([1, 1], F32, tag=name + "nmu")
                nc.vector.scalar_tensor_tensor(
                    nmu[:], mv[:, 0:1], -1.0, inv[:], ALU.mult, ALU.mult
                )
                ln0 = rows.tile([1, HID], F32, tag="ln0")
                nc.scalar.activation(
                    ln0[:], src_row[:], AF.Identity, bias=nmu[:], scale=inv[:]
                )
                pst = pe_transpose(ln0, KC_H)
                lnc = kcp.tile([P, KC_H], F32, tag=name + "c")
                nc.vector.tensor_mul(lnc[:], pst[:, :KC_H], gbc[:, gb_idx, :])
                nc.vector.tensor_add(lnc[:], lnc[:], gbc[:, gb_idx + 1, :])
                return to_lhsT(lnc[:], KC_H, name + "kc")

            def gvec(prec):
                if prec == "bf16":
                    return (xv, hv)
                if prec == "f32r":
                    return (xvr, hvr)
                return (xvf, hvf)

            # ---- stage A: gates --------------------------------------
            ig_r = rows.tile([1, HID], F32, tag="ig")
            gg_r = rows.tile([1, HID], F32, tag="gg")
            og_r = rows.tile([1, HID], F32, tag="og")
            d_r = rows.tile([1, HID], F32, tag="dr")
            rc_r = rows.tile([1, HID], F32, tag="rc")

            gate_rows = {0: ig_r, 2: gg_r, 3: og_r}
            gate_fn = {0: AF.Sigmoid, 2: AF.Tanh, 3: AF.Sigmoid}

            def emit_gate(g):
                pg = gate_prec[g]
                xvec, hvec = gvec(pg)

                def mk_consume(gr=gate_rows[g], fn=gate_fn[g]):
                    def consume(col, ps, prec):
                        nc.scalar.activation(
                            gr[:, col : col + NT], collapse(ps, prec), fn
                        )
                    return consume

                stream_group(
                    [(xvec, Wx_t[g], INP), (hvec, Wh_t[g], HID)],
                    HID, pg, mk_consume(), f"g{g}",
                )

            def consume_x(col, ps, prec):
                pc = collapse(ps, prec)
                nc.vector.tensor_copy(d_r[:, col : col + NT], pc)
                nc.vector.reciprocal(rc_r[:, col : col + NT], pc)

            def consume_h(col, ps, prec):
                nc.vector.tensor_sub(
                    d_r[:, col : col + NT],
                    d_r[:, col : col + NT],
                    collapse(ps, prec),
                )

            pg1 = gate_prec[1]
            xv1, hv1 = gvec(pg1)
            # interleave the (PE-heavier) fp32 gate-1 passes between the
            # bf16 gates so the PE hump is absorbed by the DMA prefetch bufs
            emit_gate(0)
            # small loads not needed until much later: emitted here so their
            # DMA-queue slots come after the first weight blocks (faster start)
            c0r = rows.tile([1, HID], F32, tag="c0r")
            nc.scalar.dma_start(c0r[:], c0_t[:])
            gbc = kcp.tile([P, 4, KC_H], F32, tag="gbc")
            nc.scalar.dma_start(gbc[:], gb_t[:])
            bdbr = rows.tile([1, 3, AHID], F32, tag="bdbr")
            nc.scalar.dma_start(bdbr[:], bdb_t[:])
            bilcr = rows.tile([1, AHID], F32, tag="bilcr")
            nc.scalar.dma_start(bilcr[:], bilc_t[:])
            bilhr = rows.tile([1, AHID], F32, tag="bilhr")
            nc.scalar.dma_start(bilhr[:], bilh_t[:])
            stream_group([(xv1, Wx_t[1], INP)], HID, pg1, consume_x, "g1x")
            emit_gate(2)
            stream_group([(hv1, Wh_t[1], HID)], HID, pg1, consume_h, "g1h")
            emit_gate(3)

            # ---- stage A chain: c_m, h_m, layernorms ------------------
            flush_deferred()
            with nc.named_scope("gate_chain"):
                # q = d / xg1   (in place of d)
                nc.vector.tensor_mul(d_r[:], d_r[:], rc_r[:])
                # |q|  (in place of rc)
                nc.scalar.activation(rc_r[:], d_r[:], AF.Abs)
                # |q| * c0   (into d: q dead)
                nc.vector.tensor_mul(d_r[:], rc_r[:], c0r[:])
                # ig * gg  (in place of ig)
                nc.vector.tensor_mul(ig_r[:], ig_r[:], gg_r[:])
                # fg_p*c0 = c0 - |q|*c0   (into rc: |q| dead)
                nc.vector.tensor_sub(rc_r[:], c0r[:], d_r[:])
                cm_r = rows.tile([1, HID], F32, tag="cmr")
                nc.vector.tensor_add(cm_r[:], rc_r[:], ig_r[:])
                nc.scalar.dma_start(cm_t[:], cm_r[:])

            with nc.named_scope("ln_c"):
                lc_kc = layernorm_lhsT(cm_r, 2, "lnc")

            with nc.named_scope("h_m"):
                th_r = rows.tile([1, HID], F32, tag="gg")  # reuse gg slot
                nc.scalar.activation(th_r[:], cm_r[:], AF.Tanh)
                # h_m = og * tanh(c_m)  (in place of og)
                nc.vector.tensor_mul(og_r[:], og_r[:], th_r[:])
                nc.scalar.dma_start(hm_t[:], og_r[:])

            with nc.named_scope("ln_h"):
                ht_kc = layernorm_lhsT(og_r, 0, "lnh")

            # ---- stage B ---------------------------------------------
            ct_r = rows.tile([1, AHID], F32, tag="ctr")

            def consume_ct(col, ps, prec):
                nc.vector.tensor_copy(ct_r[:, col : col + NT], collapse(ps, prec))

            stream_group([(lc_kc, Wict_t, HID)], AHID, prec_b, consume_ct, "ict")

            indb_r = rows.tile([1, AHID], F32, tag="indb")
            fndb_r = rows.tile([1, AHID], F32, tag="fndb")
            cndb_r = rows.tile([1, AHID], F32, tag="cndb")
            pre_rows = {0: indb_r, 1: fndb_r, 2: cndb_r}
            pre_fn = {0: AF.Sigmoid, 1: AF.Sigmoid, 2: AF.Tanh}

            def emit_db(j):
                def mk_consume_pre(jj=j):
                    def consume(col, ps, prec):
                        srow = stmps.tile([1, NT], F32, tag="stmp2")
                        nc.vector.tensor_add(
                            srow[:], collapse(ps, prec),
                            bdbr[:, jj, col : col + NT],
                        )
                        nc.scalar.activation(
                            pre_rows[jj][:, col : col + NT], srow[:], pre_fn[jj]
                        )
                    return consume

                stream_group(
                    [(ht_kc, Wdbx_t[j], HID), (dv, Wdbh_t[j], AHID)],
                    AHID, prec_b, mk_consume_pre(), f"db{j}",
                )

            def mk_consume_il(bias_r, out_t):
                def consume(col, ps, prec):
                    srow = stmps.tile([1, NT], F32, tag="stmp3")
                    nc.vector.tensor_add(
                        srow[:], collapse(ps, prec), bias_r[:, col : col + NT]
                    )
                    nc.scalar.dma_start(out_t[:, col : col + NT], srow[:])
                return consume

            # ---- order: db0 -> i_cell/t1 overlap db1+db2 -> c_cell/t2 ----
            emit_db(0)
            flush_deferred()
            with nc.named_scope("cells1"):
                # i_cell = in_db * c_t (in place of indb)
                nc.vector.tensor_mul(indb_r[:], indb_r[:], ct_r[:])
                ic_kc = to_lhsT(pe_transpose(indb_r, KC_A)[:, :KC_A], KC_A, "ickc")
            stream_group(
                [(ic_kc, Wilc_t, AHID)], AHID, prec_b,
                mk_consume_il(bilcr, t1_t), "t1",
            )
            emit_db(1)
            emit_db(2)
            flush_deferred()
            with nc.named_scope("cells2"):
                # fn_db * cn_db (in place of fndb)
                nc.vector.tensor_mul(fndb_r[:], fndb_r[:], cndb_r[:])
                # c_cell (in place of cndb)
                nc.vector.tensor_add(cndb_r[:], fndb_r[:], indb_r[:])
                cc_kc = to_lhsT(pe_transpose(cndb_r, KC_A)[:, :KC_A], KC_A, "cckc")
            stream_group(
                [(cc_kc, Wilh_t, AHID)], AHID, prec_b,
                mk_consume_il(bilhr, t2_t), "t2",
            )
            flush_deferred()

    _split_multi_waits(nc)
    return nc


_NC_CACHE = {}


def _get_nc(mode):
    if mode not in _NC_CACHE:
        _NC_CACHE[mode] = _build(mode)
    return _NC_CACHE[mode]


# ---------------------------------------------------------------------------
# Host side: shard, run, gather
# ---------------------------------------------------------------------------
def _bf16_split(w):
    """fp32 array -> (hi, lo) bf16 arrays with hi + lo ~= w (round-to-nearest).

    Bit-twiddled for speed: hi = RNE-round to bf16; lo = RNE(w - hi).
    """
    w = np.ascontiguousarray(w, dtype=np.float32)
    u = w.view(np.uint32)
    rhi = (u + 0x7FFF + ((u >> 16) & 1)) & 0xFFFF0000
    hi32 = rhi.view(np.float32)
    hi = (rhi >> 16).astype(np.uint16).view(ml_dtypes.bfloat16)
    lo = (w - hi32).astype(ml_dtypes.bfloat16)
    return np.ascontiguousarray(hi), np.ascontiguousarray(lo)


def _chunk_vec(v, kc):
    """[kc*128] -> [128, kc] with elem [p, c] = v[c*128 + p]."""
    return np.ascontiguousarray(np.asarray(v, np.float32).reshape(kc, P).T)


def _vec_pair(v, kc):
    hi, lo = _bf16_split(np.asarray(v, np.float32).reshape(-1))
    q = np.empty((P, kc, 2), dtype=ml_dtypes.bfloat16)
    q[:, :, 0] = np.asarray(hi).reshape(kc, P).T
    q[:, :, 1] = np.asarray(lo).reshape(kc, P).T
    return q


def _prep_core_inputs(a, mode, ins, bdb):
    f32 = np.float32
    m = {}
    gate_prec = {g: _gate_prec(mode, g) for g in range(4)}
    prec_b = _prec_b(mode)

    def put_w(name, w, prec):
        w = np.ascontiguousarray(w, dtype=f32)
        if prec == "bf16":
            hi, lo = _bf16_split(w)
            m[name + "_hi"] = hi
            m[name + "_lo"] = lo
        else:
            m[name] = w

    for g in range(4):
        put_w(f"Wx{g}", ins["Wx"][g, a], gate_prec[g])
        put_w(f"Wh{g}", ins["Wh"][g, a], gate_prec[g])
    for j in range(3):
        put_w(f"Wdbx{j}", ins["Wdbx"][j, a], prec_b)
        put_w(f"Wdbh{j}", ins["Wdbh"][j, a], prec_b)
    put_w("Wict", ins["W_ict"][a], prec_b)
    put_w("Wilc", ins["W_ilc"][a], prec_b)
    put_w("Wilh", ins["W_ilh"][a], prec_b)

    need_bf_vec = any(p == "bf16" for p in gate_prec.values())
    need_f32_vec = any(p == "f32" for p in gate_prec.values())
    need_f32r_vec = any(p == "f32r" for p in gate_prec.values())
    if need_bf_vec:
        m["xvq"] = _vec_pair(ins["in_"], KC_H)
        m["hvq"] = _vec_pair(ins["h0"][a], KC_H)
    if need_f32_vec:
        m["xvf"] = _chunk_vec(ins["in_"], KC_H)
        m["hvf"] = _chunk_vec(ins["h0"][a], KC_H)
    if need_f32r_vec:
        m["xvr"] = _chunk_vec(ins["in_"], KC_H)
        m["hvr"] = _chunk_vec(ins["h0"][a], KC_H)
    if prec_b == "bf16":
        m["dvq"] = _vec_pair(ins["hdb0"], KC_A)
    elif prec_b == "f32r":
        m["dvr"] = _chunk_vec(ins["hdb0"], KC_A)
    else:
        m["dvf"] = _chunk_vec(ins["hdb0"], KC_A)

    m["c0r"] = np.ascontiguousarray(ins["c0"][a], dtype=f32).reshape(1, HID)
    gb = np.empty((P, 4, KC_H), dtype=f32)
    gb[:, 0] = _chunk_vec(ins["ln_gamma"][0, a], KC_H)
    gb[:, 1] = _chunk_vec(ins["ln_beta"][0, a], KC_H)
    gb[:, 2] = _chunk_vec(ins["ln_gamma"][1, a], KC_H)
    gb[:, 3] = _chunk_vec(ins["ln_beta"][1, a], KC_H)
    m["gbc"] = gb
    m["bdbr"] = np.ascontiguousarray(bdb[:, a], dtype=f32).reshape(1, 3, AHID)
    m["bilcr"] = np.ascontiguousarray(ins["b_ilc"][a], dtype=f32).reshape(1, AHID)
    m["bilhr"] = np.ascontiguousarray(ins["b_ilh"][a], dtype=f32).reshape(1, AHID)
    return m


def _run(inputs, mode=None, **run_kwargs):
    mode = mode or MODE
    f32 = np.float32
    ins = {k: np.asarray(v) for k, v in inputs.items()}
    bdb = ins["bdbx"].astype(f32) + ins["bdbh"].astype(f32)

    in_maps = [_prep_core_inputs(a, mode, ins, bdb) for a in range(NAXIS)]

    nc = _get_nc(mode)
    res = run_bass_kernel_spmd(nc, in_maps, core_ids=list(range(NAXIS)), **run_kwargs)

    h_m = np.stack([res.results[a]["hm"].reshape(HID) for a in range(NAXIS)])
    c_m = np.stack([res.results[a]["cm"].reshape(HID) for a in range(NAXIS)])
    t1 = np.stack([res.results[a]["t1"].reshape(AHID) for a in range(NAXIS)])
    t2 = np.stack([res.results[a]["t2"].reshape(AHID) for a in range(NAXIS)])

    su_c = t1.astype(np.float64).sum(0)
    su_h = t2.astype(np.float64).sum(0)
    h_db = (1.0 / (1.0 + np.exp(-su_h))) * np.tanh(su_c)

    return (h_m.astype(f32), c_m.astype(f32), h_db.astype(f32)), res


def kernel(**inputs):
    out, _ = _run(inputs)
    return out


# revision 15
# speedup vs baseline: 1.1051x; 1.0172x over previous
"""Trainium2 Bass kernel for nn_Branch_Cell (branched LSTM-style cell).

Sharding: expert parallelism over the naxis dimension (naxis == 8 == n_cores).
Core `a` owns axis `a`: it streams that axis's ~189 MB of weights from HBM
(the memory roofline), computes h_m[a], c_m[a] and the partial sums
t1[a], t2[a]; the host sums t1/t2 over axes and applies the final
sigmoid*tanh (8 KB of work, avoids the collective latency floor).

Precision modes:
  'hybrid' (default): weights stream as host-decomposed bf16 hi + bf16 lo
     pairs (same total bytes as fp32; PE runs both halves at full bf16 rate
     with the vector's (hi, lo) pair as a [K, 2] stationary operand) --
     EXCEPT gate 1, whose output feeds a division (1 - |(xg1-hg1)/xg1|)
     that amplifies error near xg1 ~ 0; that gate streams exact fp32.
  'bf16x2': everything bf16 hi/lo.
  'f32': everything fp32 (PE at 4 cyc/row, slightly under HBM rate).

All matvecs keep the vector as the PE stationary operand (weights are the
moving operand), so kernel time is bounded by HBM->SBUF DMA. Weight blocks
are [512 rows x colspan cols] laid out [128p, 4kc, colspan] so every DMA
descriptor moves a 2 KB contiguous run (~95% of DMA line rate).
Elementwise gate/LN math runs on single-partition rows (ACT + DVE);
row->partition-chunk conversion for downstream lhsT operands uses tiny
K=1 transpose matmuls on the PE (no DMA scatters).
"""

import numpy as np
import ml_dtypes
from contextlib import ExitStack

import concourse.bass as bass
import concourse.tile as tile
from concourse import mybir
from concourse.bass_utils import run_bass_kernel_spmd

F32 = mybir.dt.float32
F32R = mybir.dt.float32r
BF16 = mybir.dt.bfloat16
AF = mybir.ActivationFunctionType
ALU = mybir.AluOpType

P = 128
INP = 2048
HID = 2048
AHID = 1024
NAXIS = 8
NT = 512                 # matmul moving free dim
KC_H = HID // P          # 16
KC_A = AHID // P         # 8

MODE = "hybrid"          # "hybrid" | "hybrid_r" | "f32r" | "bf16x2" | "f32"

# Weight dtypes per precision name
_WDT = None  # set below after mybir import resolution

import os as _os
if _os.environ.get("LDWOPT", "0") == "1":
    # walrus's own default is --enable-ldw-opt=true; bass pins it false.
    # Allow re-enabling for A/B (validated against the f64 oracle).
    import concourse.bass_utils as _bu
    _orig_run_command = _bu.run_command
    def _run_command_ldwopt(argv, **kw):
        argv = ["--enable-ldw-opt=true" if a == "--enable-ldw-opt=false" else a
                for a in argv]
        return _orig_run_command(argv, **kw)
    _bu.run_command = _run_command_ldwopt

EPS = 1e-5


# ---------------------------------------------------------------------------
# Workaround: the nix walrus in this container allows only ONE sync wait per
# non-EventSemaphore instruction ("Too many sync wait commands" in
# setupSyncWait). Tile's kernel-tail drain (and occasionally other insts)
# carries several. Split the extras onto single-wait NoOps placed just before
# the instruction on the same engine (per-engine program order preserved).
# ---------------------------------------------------------------------------
def _split_multi_waits(nc):
    n_new = 0
    for f in nc.m.functions:
        for blk in f.blocks:
            out = []
            for inst in blk.instructions:
                si = inst.sync_info
                waits = list(si.on_wait) if si is not None else []
                if len(waits) > 1 and inst.opcode != "EventSemaphore":
                    for w in waits[:-1]:
                        n_new += 1
                        out.append(
                            mybir.InstNoOp(
                                name=f"{inst.name}-wsplit{n_new}",
                                engine=inst.engine,
                                debug=inst.debug,
                                ins=[],
                                outs=[],
                                sync_info=mybir.SyncInfo(on_update=[], on_wait=[w]),
                            )
                        )
                    si.on_wait = [waits[-1]]
                    inst.sync_info = si
                out.append(inst)
            blk.instructions = out
    return n_new


def _gate_prec(mode, g):
    if mode == "f32":
        return "f32"
    if mode == "f32r":
        return "f32r"
    if mode in ("hybrid", "hybrid_r") and g == 1:
        return "f32"
    return "f32r" if mode == "hybrid_r" else "bf16"


def _prec_b(mode):
    if mode == "f32":
        return "f32"
    if mode in ("f32r", "hybrid_r"):
        return "f32r"
    return "bf16"


# ---------------------------------------------------------------------------
# Kernel builder
# ---------------------------------------------------------------------------
def _build(mode):
    nc = bass.Bass()
    prec_b = _prec_b(mode)

    def din(name, shape, dt=F32):
        return nc.dram_tensor(name, shape, dt, kind="ExternalInput")

    def dout(name, shape, dt=F32):
        return nc.dram_tensor(name, shape, dt, kind="ExternalOutput")

    def w_pair(name, shape, prec):
        """Declare weight tensor(s): (hi_handle, lo_handle|None)."""
        if prec == "bf16":
            return din(name + "_hi", shape, BF16), din(name + "_lo", shape, BF16)
        return din(name, shape, F32 if prec == "f32" else F32R), None

    gate_prec = {g: _gate_prec(mode, g) for g in range(4)}
    Wx_t = {g: w_pair(f"Wx{g}", [INP, HID], gate_prec[g]) for g in range(4)}
    Wh_t = {g: w_pair(f"Wh{g}", [HID, HID], gate_prec[g]) for g in range(4)}
    Wdbx_t = {j: w_pair(f"Wdbx{j}", [HID, AHID], prec_b) for j in range(3)}
    Wdbh_t = {j: w_pair(f"Wdbh{j}", [AHID, AHID], prec_b) for j in range(3)}
    Wict_t = w_pair("Wict", [HID, AHID], prec_b)
    Wilc_t = w_pair("Wilc", [AHID, AHID], prec_b)
    Wilh_t = w_pair("Wilh", [AHID, AHID], prec_b)

    need_bf_vec = any(p == "bf16" for p in gate_prec.values())
    need_f32_vec = any(p == "f32" for p in gate_prec.values())
    need_f32r_vec = any(p == "f32r" for p in gate_prec.values())

    xv_t = din("xvq", [P, KC_H, 2], BF16) if need_bf_vec else None
    hv_t = din("hvq", [P, KC_H, 2], BF16) if need_bf_vec else None
    xvf_t = din("xvf", [P, KC_H], F32) if need_f32_vec else None
    hvf_t = din("hvf", [P, KC_H], F32) if need_f32_vec else None
    xvr_t = din("xvr", [P, KC_H], F32R) if need_f32r_vec else None
    hvr_t = din("hvr", [P, KC_H], F32R) if need_f32r_vec else None
    if prec_b == "bf16":
        dv_t = din("dvq", [P, KC_A, 2], BF16)
    elif prec_b == "f32r":
        dv_t = din("dvr", [P, KC_A], F32R)
    else:
        dv_t = din("dvf", [P, KC_A], F32)

    c0_t = din("c0r", [1, HID])
    gb_t = din("gbc", [P, 4, KC_H])      # g0,b0,g1,b1 chunked [p, i, kc]
    bdb_t = din("bdbr", [1, 3, AHID])    # bdbx[:,a] + bdbh[:,a]
    bilc_t = din("bilcr", [1, AHID])
    bilh_t = din("bilhr", [1, AHID])

    hm_t = dout("hm", [1, HID])
    cm_t = dout("cm", [1, HID])
    t1_t = dout("t1", [1, AHID])
    t2_t = dout("t2", [1, AHID])

    with tile.TileContext(nc) as tc:
        with ExitStack() as ctx:
            wpool = ctx.enter_context(tc.tile_pool(name="wstream", bufs=6))
            rows = ctx.enter_context(tc.tile_pool(name="rows", bufs=1))
            stmps = ctx.enter_context(tc.tile_pool(name="stmps", bufs=3))
            kcp = ctx.enter_context(tc.tile_pool(name="kcp", bufs=1))
            sm = ctx.enter_context(tc.tile_pool(name="smalls", bufs=1))
            pmm = ctx.enter_context(tc.tile_pool(name="pmm", bufs=4, space="PSUM"))
            ptr = ctx.enter_context(tc.tile_pool(name="ptr", bufs=2, space="PSUM"))
            pcol = ctx.enter_context(tc.tile_pool(name="pcol", bufs=2, space="PSUM"))

            # ---- small input loads -----------------------------------
            xv = hv = xvf = hvf = xvr = hvr = None
            if need_bf_vec:
                xv = kcp.tile([P, KC_H, 2], BF16, tag="xv")
                hv = kcp.tile([P, KC_H, 2], BF16, tag="hv")
                nc.sync.dma_start(xv[:], xv_t[:])
                nc.sync.dma_start(hv[:], hv_t[:])
            if need_f32_vec:
                xvf = kcp.tile([P, KC_H], F32, tag="xvf")
                hvf = kcp.tile([P, KC_H], F32, tag="hvf")
                nc.sync.dma_start(xvf[:], xvf_t[:])
                nc.sync.dma_start(hvf[:], hvf_t[:])
            if need_f32r_vec:
                xvr = kcp.tile([P, KC_H], F32R, tag="xvr")
                hvr = kcp.tile([P, KC_H], F32R, tag="hvr")
                nc.sync.dma_start(xvr[:], xvr_t[:])
                nc.sync.dma_start(hvr[:], hvr_t[:])
            if prec_b == "bf16":
                dv = kcp.tile([P, KC_A, 2], BF16, tag="dv")
            elif prec_b == "f32r":
                dv = kcp.tile([P, KC_A], F32R, tag="dv")
            else:
                dv = kcp.tile([P, KC_A], F32, tag="dv")
            nc.sync.dma_start(dv[:], dv_t[:])

            ones = sm.tile([1, 1], F32, tag="ones")
            nc.vector.memset(ones[:], 1.0)
            ones2 = sm.tile([2, 1], F32, tag="ones2")
            nc.vector.memset(ones2[:], 1.0)

            # ---- helpers ---------------------------------------------
            def collapse(ps, prec):
                """psum [M, 512] -> psum row [1, 512] AP (hi+lo summed).

                PSUM reads must start at partition 0 and a DVE op may read
                only one PSUM operand, so the two rows are combined on the
                PE: copy [2, NT] to SBUF, then ones2.T @ t sums the rows.
                """
                if prec != "bf16":
                    return ps[0:1, :]
                t = stmps.tile([2, NT], F32, tag="clp")
                nc.vector.tensor_copy(t[:], ps[:])
                pc = pcol.tile([1, NT], F32, tag="pcol")
                nc.tensor.matmul(pc[:], ones2[0:2, 0:1], t[:], start=True, stop=True)
                return pc

            # Consume-stage software pipeline: each column-block's psum
            # collapse/activation is deferred until the NEXT block's matmuls
            # have been emitted, so the in-order PE never stalls on the DVE
            # copy feeding the collapse matmul.
            deferred = []

            def flush_deferred():
                while deferred:
                    deferred.pop(0)()

            def stream_group(contribs, w_cols, prec, consume, scope):
                """Sum_i vec_i.T @ W_i over full K, streamed in row-contiguous
                blocks; calls consume(col_off, ps, prec) per 512-col output
                (deferred by one block).

                contribs: list of (vec_tile, (w_hi, w_lo), K).
                Weight blocks are [512, colspan] -> [128, 4, colspan] tiles
                (every DMA descriptor run = colspan * dtype = 2 KB).
                """
                colspan = 1024 if prec == "bf16" else 512
                n_ct = colspan // NT
                m_dim = 2 if prec == "bf16" else 1
                n_halves = 2 if prec == "bf16" else 1
                wdt = {"bf16": BF16, "f32": F32, "f32r": F32R}[prec]
                wtag = "wblk"
                wbufs = 8
                total_per_ct = sum((K // NT) * 4 * n_halves for _, _, K in contribs)

                for cb in range(w_cols // colspan):
                    with nc.named_scope(f"{scope}_cb{cb}"):
                        pss = [
                            pmm.tile([m_dim, NT], F32, tag="mmps", name="mmps")
                            for _ in range(n_ct)
                        ]
                        cnt = 0
                        n_blk = 0
                        for vec, (w_hi, w_lo), K in contribs:
                            for kb in range(K // NT):
                                for w_t in (w_hi, w_lo)[:n_halves]:
                                    blk = wpool.tile(
                                        [P, 4, colspan], wdt, tag=wtag, bufs=wbufs
                                    )
                                    nc.sync.dma_start(
                                        blk[:],
                                        w_t[
                                            kb * NT : (kb + 1) * NT,
                                            cb * colspan : (cb + 1) * colspan,
                                        ].rearrange("(kc p) n -> p kc n", p=P),
                                    )
                                    for kc in range(4):
                                        ki = kb * 4 + kc
                                        lhsT = (
                                            vec[:, ki, :]
                                            if prec == "bf16"
                                            else vec[:, ki : ki + 1]
                                        )
                                        for ct in range(n_ct):
                                            nc.tensor.matmul(
                                                pss[ct][:],
                                                lhsT,
                                                blk[:, kc, ct * NT : (ct + 1) * NT],
                                                start=(cnt // n_ct == 0),
                                                stop=(cnt // n_ct == total_per_ct - 1),
                                            )
                                            cnt += 1
                                    n_blk += 1
                                    if n_blk == 1:
                                        flush_deferred()

                        def _consume_cb(cb=cb, pss=pss):
                            for ct in range(n_ct):
                                consume(cb * colspan + ct * NT, pss[ct], prec)

                        deferred.append(_consume_cb)

            def pe_transpose(row_ap, cpt):
                """[1, cpt*128] f32 row -> psum tile [P, cpt] f32."""
                pst = ptr.tile([P, KC_H], F32, tag="ptr")
                for c in range(cpt):
                    nc.tensor.matmul(
                        pst[:, c : c + 1],
                        row_ap[0:1, c * P : (c + 1) * P],
                        ones[0:1, 0:1],
                        start=True,
                        stop=True,
                    )
                return pst

            def to_lhsT(src_ap, cpt, name):
                """[P, cpt] f32 (psum/sbuf) -> lhsT tile for stage-B matvecs."""
                if prec_b == "bf16":
                    tq = kcp.tile([P, cpt, 2], BF16, tag=name)
                    nc.vector.tensor_copy(tq[:, :, 0], src_ap)
                    hi32 = kcp.tile([P, cpt], F32, tag=name + "32")
                    nc.vector.tensor_copy(hi32[:], tq[:, :, 0])
                    nc.vector.tensor_sub(tq[:, :, 1], src_ap, hi32[:])
                    return tq
                wdt = F32R if prec_b == "f32r" else F32
                tq = kcp.tile([P, cpt], wdt, tag=name)
                nc.vector.tensor_copy(tq[:], src_ap)
                return tq

            def layernorm_lhsT(src_row, gb_idx, name):
                """LN over [1, HID] row; gamma/beta applied after the
                row->[P, KC_H] transpose. Returns stage-B lhsT tile."""
                stats = sm.tile([1, HID // 512, 6], F32, tag=name + "st")
                for c_ in range(HID // 512):
                    nc.vector.bn_stats(
                        stats[:, c_, :], src_row[:, c_ * 512 : (c_ + 1) * 512]
                    )
                mv = sm.tile([1, 2], F32, tag=name + "mv")
                nc.vector.bn_aggr(mv[:], stats[:])
                vs = sm.tile([1, 1], F32, tag=name + "vs")
                nc.vector.tensor_scalar_add(vs[:], mv[:, 1:2], EPS)
                sd = sm.tile([1, 1], F32, tag=name + "sd")
                nc.scalar.sqrt(sd[:], vs[:])
                inv = sm.tile([1, 1], F32, tag=name + "inv")
                nc.vector.reciprocal(inv[:], sd[:])
                nmu = sm.tile([1, 1], F32, tag=name + "nmu")
                nc.vector.scalar_tensor_tensor(
                    nmu[:], mv[:, 0:1], -1.0, inv[:], ALU.mult, ALU.mult
                )
                ln0 = rows.tile([1, HID], F32, tag="ln0")
                nc.scalar.activation(
                    ln0[:], src_row[:], AF.Identity, bias=nmu[:], scale=inv[:]
                )
                pst = pe_transpose(ln0, KC_H)
                lnc = kcp.tile([P, KC_H], F32, tag=name + "c")
                nc.vector.tensor_mul(lnc[:], pst[:, :KC_H], gbc[:, gb_idx, :])
                nc.vector.tensor_add(lnc[:], lnc[:], gbc[:, gb_idx + 1, :])
                return to_lhsT(lnc[:], KC_H, name + "kc")

            def gvec(prec):
                if prec == "bf16":
                    return (xv, hv)
                if prec == "f32r":
                    return (xvr, hvr)
                return (xvf, hvf)

            # ---- stage A: gates --------------------------------------
            ig_r = rows.tile([1, HID], F32, tag="ig")
            gg_r = rows.tile([1, HID], F32, tag="gg")
            og_r = rows.tile([1, HID], F32, tag="og")
            d_r = rows.tile([1, HID], F32, tag="dr")
            rc_r = rows.tile([1, HID], F32, tag="rc")

            gate_rows = {0: ig_r, 2: gg_r, 3: og_r}
            gate_fn = {0: AF.Sigmoid, 2: AF.Tanh, 3: AF.Sigmoid}

            def emit_gate(g):
                pg = gate_prec[g]
                xvec, hvec = gvec(pg)

                def mk_consume(gr=gate_rows[g], fn=gate_fn[g]):
                    def consume(col, ps, prec):
                        nc.scalar.activation(
                            gr[:, col : col + NT], collapse(ps, prec), fn
                        )
                    return consume

                stream_group(
                    [(xvec, Wx_t[g], INP), (hvec, Wh_t[g], HID)],
                    HID, pg, mk_consume(), f"g{g}",
                )

            def consume_x(col, ps, prec):
                pc = collapse(ps, prec)
                nc.vector.tensor_copy(d_r[:, col : col + NT], pc)
                nc.vector.reciprocal(rc_r[:, col : col + NT], pc)

            def consume_h(col, ps, prec):
                nc.vector.tensor_sub(
                    d_r[:, col : col + NT],
                    d_r[:, col : col + NT],
                    collapse(ps, prec),
                )

            pg1 = gate_prec[1]
            xv1, hv1 = gvec(pg1)
            # interleave the (PE-heavier) fp32 gate-1 passes between the
            # bf16 gates so the PE hump is absorbed by the DMA prefetch bufs
            emit_gate(0)
            # small loads not needed until much later: emitted here so their
            # DMA-queue slots come after the first weight blocks (faster start)
            c0r = rows.tile([1, HID], F32, tag="c0r")
            nc.sync.dma_start(c0r[:], c0_t[:])
            gbc = kcp.tile([P, 4, KC_H], F32, tag="gbc")
            nc.sync.dma_start(gbc[:], gb_t[:])
            bdbr = rows.tile([1, 3, AHID], F32, tag="bdbr")
            nc.sync.dma_start(bdbr[:], bdb_t[:])
            bilcr = rows.tile([1, AHID], F32, tag="bilcr")
            nc.sync.dma_start(bilcr[:], bilc_t[:])
            bilhr = rows.tile([1, AHID], F32, tag="bilhr")
            nc.sync.dma_start(bilhr[:], bilh_t[:])
            stream_group([(xv1, Wx_t[1], INP)], HID, pg1, consume_x, "g1x")
            emit_gate(2)
            stream_group([(hv1, Wh_t[1], HID)], HID, pg1, consume_h, "g1h")
            emit_gate(3)

            # ---- stage A chain: c_m, h_m, layernorms ------------------
            flush_deferred()
            with nc.named_scope("gate_chain"):
                # q = d / xg1   (in place of d)
                nc.vector.tensor_mul(d_r[:], d_r[:], rc_r[:])
                # |q|  (in place of rc)
                nc.scalar.activation(rc_r[:], d_r[:], AF.Abs)
                # |q| * c0   (into d: q dead)
                nc.vector.tensor_mul(d_r[:], rc_r[:], c0r[:])
                # ig * gg  (in place of ig)
                nc.vector.tensor_mul(ig_r[:], ig_r[:], gg_r[:])
                # fg_p*c0 = c0 - |q|*c0   (into rc: |q| dead)
                nc.vector.tensor_sub(rc_r[:], c0r[:], d_r[:])
                cm_r = rows.tile([1, HID], F32, tag="cmr")
                nc.vector.tensor_add(cm_r[:], rc_r[:], ig_r[:])
                nc.sync.dma_start(cm_t[:], cm_r[:])

            with nc.named_scope("ln_c"):
                lc_kc = layernorm_lhsT(cm_r, 2, "lnc")

            with nc.named_scope("h_m"):
                th_r = rows.tile([1, HID], F32, tag="gg")  # reuse gg slot
                nc.scalar.activation(th_r[:], cm_r[:], AF.Tanh)
                # h_m = og * tanh(c_m)  (in place of og)
                nc.vector.tensor_mul(og_r[:], og_r[:], th_r[:])
                nc.sync.dma_start(hm_t[:], og_r[:])

            with nc.named_scope("ln_h"):
                ht_kc = layernorm_lhsT(og_r, 0, "lnh")

            # ---- stage B ---------------------------------------------
            ct_r = rows.tile([1, AHID], F32, tag="ctr")

            def consume_ct(col, ps, prec):
                nc.vector.tensor_copy(ct_r[:, col : col + NT], collapse(ps, prec))

            stream_group([(lc_kc, Wict_t, HID)], AHID, prec_b, consume_ct, "ict")

            indb_r = rows.tile([1, AHID], F32, tag="indb")
            fndb_r = rows.tile([1, AHID], F32, tag="fndb")
            cndb_r = rows.tile([1, AHID], F32, tag="cndb")
            pre_rows = {0: indb_r, 1: fndb_r, 2: cndb_r}
            pre_fn = {0: AF.Sigmoid, 1: AF.Sigmoid, 2: AF.Tanh}

            def emit_db(j):
                def mk_consume_pre(jj=j):
                    def consume(col, ps, prec):
                        srow = stmps.tile([1, NT], F32, tag="stmp2")
                        nc.vector.tensor_add(
                            srow[:], collapse(ps, prec),
                            bdbr[:, jj, col : col + NT],
                        )
                        nc.scalar.activation(
                            pre_rows[jj][:, col : col + NT], srow[:], pre_fn[jj]
                        )
                    return consume

                stream_group(
                    [(ht_kc, Wdbx_t[j], HID), (dv, Wdbh_t[j], AHID)],
                    AHID, prec_b, mk_consume_pre(), f"db{j}",
                )

            def mk_consume_il(bias_r, out_t):
                def consume(col, ps, prec):
                    srow = stmps.tile([1, NT], F32, tag="stmp3")
                    nc.vector.tensor_add(
                        srow[:], collapse(ps, prec), bias_r[:, col : col + NT]
                    )
                    nc.sync.dma_start(out_t[:, col : col + NT], srow[:])
                return consume

            # ---- order: db0 -> i_cell/t1 overlap db1+db2 -> c_cell/t2 ----
            emit_db(0)
            flush_deferred()
            with nc.named_scope("cells1"):
                # i_cell = in_db * c_t (in place of indb)
                nc.vector.tensor_mul(indb_r[:], indb_r[:], ct_r[:])
                ic_kc = to_lhsT(pe_transpose(indb_r, KC_A)[:, :KC_A], KC_A, "ickc")
            stream_group(
                [(ic_kc, Wilc_t, AHID)], AHID, prec_b,
                mk_consume_il(bilcr, t1_t), "t1",
            )
            emit_db(1)
            emit_db(2)
            flush_deferred()
            with nc.named_scope("cells2"):
                # fn_db * cn_db (in place of fndb)
                nc.vector.tensor_mul(fndb_r[:], fndb_r[:], cndb_r[:])
                # c_cell (in place of cndb)
                nc.vector.tensor_add(cndb_r[:], fndb_r[:], indb_r[:])
                cc_kc = to_lhsT(pe_transpose(cndb_r, KC_A)[:, :KC_A], KC_A, "cckc")
            stream_group(
                [(cc_kc, Wilh_t, AHID)], AHID, prec_b,
                mk_consume_il(bilhr, t2_t), "t2",
            )
            flush_deferred()

    _split_multi_waits(nc)
    return nc


_NC_CACHE = {}


def _get_nc(mode):
    if mode not in _NC_CACHE:
        _NC_CACHE[mode] = _build(mode)
    return _NC_CACHE[mode]


# ---------------------------------------------------------------------------
# Host side: shard, run, gather
# ---------------------------------------------------------------------------
def _bf16_split(w):
    """fp32 array -> (hi, lo) bf16 arrays with hi + lo ~= w (round-to-nearest).

    Bit-twiddled for speed: hi = RNE-round to bf16; lo = RNE(w - hi).
    """
    w = np.ascontiguousarray(w, dtype=np.float32)
    u = w.view(np.uint32)
    rhi = (u + 0x7FFF + ((u >> 16) & 1)) & 0xFFFF0000
    hi32 = rhi.view(np.float32)
    hi = (rhi >> 16).astype(np.uint16).view(ml_dtypes.bfloat16)
    lo = (w - hi32).astype(ml_dtypes.bfloat16)
    return np.ascontiguousarray(hi), np.ascontiguousarray(lo)


def _chunk_vec(v, kc):
    """[kc*128] -> [128, kc] with elem [p, c] = v[c*128 + p]."""
    return np.ascontiguousarray(np.asarray(v, np.float32).reshape(kc, P).T)


def _vec_pair(v, kc):
    hi, lo = _bf16_split(np.asarray(v, np.float32).reshape(-1))
    q = np.empty((P, kc, 2), dtype=ml_dtypes.bfloat16)
    q[:, :, 0] = np.asarray(hi).reshape(kc, P).T
    q[:, :, 1] = np.asarray(lo).reshape(kc, P).T
    return q


def _prep_core_inputs(a, mode, ins, bdb):
    f32 = np.float32
    m = {}
    gate_prec = {g: _gate_prec(mode, g) for g in range(4)}
    prec_b = _prec_b(mode)

    def put_w(name, w, prec):
        w = np.ascontiguousarray(w, dtype=f32)
        if prec == "bf16":
            hi, lo = _bf16_split(w)
            m[name + "_hi"] = hi
            m[name + "_lo"] = lo
        else:
            m[name] = w

    for g in range(4):
        put_w(f"Wx{g}", ins["Wx"][g, a], gate_prec[g])
        put_w(f"Wh{g}", ins["Wh"][g, a], gate_prec[g])
    for j in range(3):
        put_w(f"Wdbx{j}", ins["Wdbx"][j, a], prec_b)
        put_w(f"Wdbh{j}", ins["Wdbh"][j, a], prec_b)
    put_w("Wict", ins["W_ict"][a], prec_b)
    put_w("Wilc", ins["W_ilc"][a], prec_b)
    put_w("Wilh", ins["W_ilh"][a], prec_b)

    need_bf_vec = any(p == "bf16" for p in gate_prec.values())
    need_f32_vec = any(p == "f32" for p in gate_prec.values())
    need_f32r_vec = any(p == "f32r" for p in gate_prec.values())
    if need_bf_vec:
        m["xvq"] = _vec_pair(ins["in_"], KC_H)
        m["hvq"] = _vec_pair(ins["h0"][a], KC_H)
    if need_f32_vec:
        m["xvf"] = _chunk_vec(ins["in_"], KC_H)
        m["hvf"] = _chunk_vec(ins["h0"][a], KC_H)
    if need_f32r_vec:
        m["xvr"] = _chunk_vec(ins["in_"], KC_H)
        m["hvr"] = _chunk_vec(ins["h0"][a], KC_H)
    if prec_b == "bf16":
        m["dvq"] = _vec_pair(ins["hdb0"], KC_A)
    elif prec_b == "f32r":
        m["dvr"] = _chunk_vec(ins["hdb0"], KC_A)
    else:
        m["dvf"] = _chunk_vec(ins["hdb0"], KC_A)

    m["c0r"] = np.ascontiguousarray(ins["c0"][a], dtype=f32).reshape(1, HID)
    gb = np.empty((P, 4, KC_H), dtype=f32)
    gb[:, 0] = _chunk_vec(ins["ln_gamma"][0, a], KC_H)
    gb[:, 1] = _chunk_vec(ins["ln_beta"][0, a], KC_H)
    gb[:, 2] = _chunk_vec(ins["ln_gamma"][1, a], KC_H)
    gb[:, 3] = _chunk_vec(ins["ln_beta"][1, a], KC_H)
    m["gbc"] = gb
    m["bdbr"] = np.ascontiguousarray(bdb[:, a], dtype=f32).reshape(1, 3, AHID)
    m["bilcr"] = np.ascontiguousarray(ins["b_ilc"][a], dtype=f32).reshape(1, AHID)
    m["bilhr"] = np.ascontiguousarray(ins["b_ilh"][a], dtype=f32).reshape(1, AHID)
    return m


def _run(inputs, mode=None, **run_kwargs):
    mode = mode or MODE
    f32 = np.float32
    ins = {k: np.asarray(v) for k, v in inputs.items()}
    bdb = ins["bdbx"].astype(f32) + ins["bdbh"].astype(f32)

    in_maps = [_prep_core_inputs(a, mode, ins, bdb) for a in range(NAXIS)]

    nc = _get_nc(mode)
    res = run_bass_kernel_spmd(nc, in_maps, core_ids=list(range(NAXIS)), **run_kwargs)

    h_m = np.stack([res.results[a]["hm"].reshape(HID) for a in range(NAXIS)])
    c_m = np.stack([res.results[a]["cm"].reshape(HID) for a in range(NAXIS)])
    t1 = np.stack([res.results[a]["t1"].reshape(AHID) for a in range(NAXIS)])
    t2 = np.stack([res.results[a]["t2"].reshape(AHID) for a in range(NAXIS)])

    su_c = t1.astype(np.float64).sum(0)
    su_h = t2.astype(np.float64).sum(0)
    h_db = (1.0 / (1.0 + np.exp(-su_h))) * np.tanh(su_c)

    return (h_m.astype(f32), c_m.astype(f32), h_db.astype(f32)), res


def kernel(**inputs):
    out, _ = _run(inputs)
    return out
